# revision 11
# baseline (speedup 1.0000x reference)
"""Bayesian linear layer on 8 Trainium2 NeuronCores (Bass/Tile).

Computes out = einsum('bi,bio->bo', x, mean + W * softplus(log_std)) + bias
for B=512, D_in=D_out=512, data-parallel over the batch dim across 8 cores
(64 batches/core). The problem is HBM-bound: the three [512,512,512]
weight tensors dominate; everything else is noise.

Inputs are staged to HBM as fp16 (halves the HBM traffic; every value is
well inside fp16 range and the ~2^-12 quantization keeps the output error
around 1e-4 of absmax). All on-device arithmetic (softplus, mul, add) runs
on ACT/DVE, which compute in fp32 internally; PSUM accumulates fp32; the
output is exact fp32.

Per-core kernel, per group of PB=4 local batches:
  - DMA W/mean/log_std[b0:b0+4] (2 MB each in fp16) as [128, 8192] tiles,
    row of batch bb = 4p + r (8 KB contiguous per partition per batch).
  - softplus via Exp then Ln(.+1) on ACT (one shared table set), then
    Ws = mean + W*sp with two in-place DVE ops (fp16 at 2x rate).
  - per batch, 4 matmuls (K=128, M=64, N=512) with a masked fp16
    stationary [128, 64] holding x[b, 4p+r] in column b only,
    accumulating into one PSUM tile [64, 512] so batch b's row lands on
    PSUM partition b. Bias enters as the accumulation group's opening
    matmul: ones[1,64].T @ bias[1,512].
  - One PSUM->SBUF copy + one output DMA at the end.
"""
import sys

if "/opt/trn_rl_repo" not in sys.path:
    sys.path.insert(0, "/opt/trn_rl_repo")

import numpy as np

BATCH, D_IN, D_OUT = 512, 512, 512
N_CORES = 8
B_LOC = BATCH // N_CORES  # 64
R = 4  # rows of W per partition: i = R*p + r
P = 128
PB = 4  # batches per DMA/tile
BUFS = 3

TRACE = False  # test harness sets kernel.TRACE = True for NTFF profiling
LAST_RESULT = None  # BassKernelResults of the most recent run

_NC_CACHE = {}


def _build_nc():
    import concourse.bacc as bacc
    import concourse.mybir as mybir
    import concourse.tile as tile
    from concourse.bass import MemorySpace

    f32 = mybir.dt.float32
    f16 = mybir.dt.float16
    nc = bacc.Bacc("TRN2", target_bir_lowering=False, debug=False)
    W_d = nc.dram_tensor("w", [B_LOC, D_IN, D_OUT], f16, kind="ExternalInput")
    M_d = nc.dram_tensor("mean", [B_LOC, D_IN, D_OUT], f16, kind="ExternalInput")
    S_d = nc.dram_tensor("log_std", [B_LOC, D_IN, D_OUT], f16, kind="ExternalInput")
    X_d = nc.dram_tensor("x_t", [P, B_LOC * R], f16, kind="ExternalInput")
    Bias_d = nc.dram_tensor("bias", [1, D_OUT], f16, kind="ExternalInput")
    O_d = nc.dram_tensor("out", [B_LOC, D_OUT], f32, kind="ExternalOutput")

    with tile.TileContext(nc) as tc:
        with (
            tc.tile_pool(name="const", bufs=1) as const_pool,
            tc.tile_pool(name="big", bufs=BUFS) as big_pool,
            tc.tile_pool(name="mask", bufs=4) as mask_pool,
            tc.tile_pool(name="psum", bufs=1, space=MemorySpace.PSUM) as psum_pool,
        ):
            x_sb = const_pool.tile([P, B_LOC * R], f16)
            nc.sync.dma_start(x_sb[:], X_d[:])
            bias_sb = const_pool.tile([1, D_OUT], f16)
            nc.sync.dma_start(bias_sb[:], Bias_d[:])
            ones_sb = const_pool.tile([1, B_LOC], f16)
            nc.vector.memset(ones_sb[:], 1.0)
            out_sb = const_pool.tile([B_LOC, D_OUT], f32)

            psum_t = psum_pool.tile([B_LOC, D_OUT], f32)
            nc.tensor.matmul(
                psum_t[:], ones_sb[:], bias_sb[:], start=True, stop=False
            )

            # group sizes: PB-wide for the bulk, 1-wide for the last
            # TAIL_1W batches so the end-of-kernel softplus chain (which
            # cannot overlap any remaining DMA) is short
            TAIL_1W = 8
            groups = []
            b = 0
            while b < B_LOC - TAIL_1W:
                groups.append((b, PB))
                b += PB
            while b < B_LOC:
                groups.append((b, 1))
                b += 1

            for b0, gw in groups:
                w_t = big_pool.tile([P, PB * R * D_OUT], f16, tag="w", name="w_t")[
                    :, : gw * R * D_OUT
                ]
                m_t = big_pool.tile([P, PB * R * D_OUT], f16, tag="m", name="m_t")[
                    :, : gw * R * D_OUT
                ]
                s_t = big_pool.tile([P, PB * R * D_OUT], f16, tag="s", name="s_t")[
                    :, : gw * R * D_OUT
                ]
                src = slice(b0, b0 + gw)

                def _src(T):
                    return (
                        T[src]
                        .rearrange("b (p r) o -> b p (r o)", p=P)
                        .rearrange("b p f -> p b f")
                    )

                def _dst(t):
                    return t.rearrange("p (b f) -> p b f", b=gw)

                nc.sync.dma_start(_dst(w_t), _src(W_d))
                nc.sync.dma_start(_dst(m_t), _src(M_d))
                nc.sync.dma_start(_dst(s_t), _src(S_d))
                # softplus(z) = ln(exp(z) + 1); Exp and Ln share one ACT table set
                nc.scalar.activation(s_t, s_t, mybir.ActivationFunctionType.Exp)
                nc.scalar.activation(
                    s_t, s_t, mybir.ActivationFunctionType.Ln, bias=1.0
                )
                nc.vector.tensor_mul(w_t, w_t, s_t)
                nc.vector.tensor_add(w_t, w_t, m_t)

                for bb in range(gw):
                    b = b0 + bb
                    mask_t = mask_pool.tile([P, R * B_LOC], f16)
                    nc.vector.memset(mask_t[:], 0.0)
                    nc.vector.tensor_copy(
                        mask_t[:, b::B_LOC], x_sb[:, b * R : (b + 1) * R]
                    )
                    for r in range(R):
                        nc.tensor.matmul(
                            psum_t[:],
                            mask_t[:, r * B_LOC : (r + 1) * B_LOC],
                            w_t[
                                :, (bb * R + r) * D_OUT : (bb * R + r + 1) * D_OUT
                            ],
                            start=False,
                            stop=(b == B_LOC - 1 and r == R - 1),
                        )
            nc.vector.tensor_copy(out_sb[:], psum_t[:])
            nc.sync.dma_start(O_d[:], out_sb[:])
    nc.compile()
    return nc


def kernel(x, W, mean, log_std, bias):
    global LAST_RESULT
    from concourse.bass_utils import run_bass_kernel_spmd

    x = np.ascontiguousarray(np.asarray(x, dtype=np.float32))
    W = np.asarray(W)
    mean = np.asarray(mean)
    log_std = np.asarray(log_std)
    bias = np.asarray(bias, dtype=np.float16).reshape(1, D_OUT)

    if "nc" not in _NC_CACHE:
        _NC_CACHE["nc"] = _build_nc()
    nc = _NC_CACHE["nc"]

    in_maps = []
    for c in range(N_CORES):
        sl = slice(c * B_LOC, (c + 1) * B_LOC)
        x_c = x[sl]  # [B_LOC, D_IN]
        # x_t[p, b*R + r] = x_c[b, R*p + r]
        x_t = np.ascontiguousarray(
            x_c.reshape(B_LOC, P, R).transpose(1, 0, 2).reshape(P, B_LOC * R)
        ).astype(np.float16)
        in_maps.append(
            {
                "w": np.ascontiguousarray(W[sl]).astype(np.float16),
                "mean": np.ascontiguousarray(mean[sl]).astype(np.float16),
                "log_std": np.ascontiguousarray(log_std[sl]).astype(np.float16),
                "x_t": x_t,
                "bias": bias,
            }
        )

    res = run_bass_kernel_spmd(
        nc, in_maps, core_ids=list(range(N_CORES)), trace=TRACE
    )
    LAST_RESULT = res
    out = np.concatenate([r["out"] for r in res.results], axis=0)
    return out.astype(np.float32)
